# revision 1
# baseline (speedup 1.0000x reference)
"""Trainium2 Bass kernel for CrossAttentionAssociation.

Model: cross-attention (detections query tracks) + residual + LayerNorm,
then a pairwise association scorer:
  out[b,i,j] = sigmoid(w2 . relu(W1 (xn[b,i] * trk[b,j]) + b1) + b2)

Sharding (8 cores): core c handles batch b = c // 2 and detection rows
[256*(c%2), 256*(c%2)+256).  Tracks are replicated per batch.

Key device-side structure (per core):
- all matmuls in float32r (full-rate fp32 streaming mode, ~1.6e-4 rel err)
- attention computed feature-major per head; softmax without max-subtraction
  (scores are O(+-6), exp is exact-safe in fp32)
- association: per detection i, A_i = w1T * xn_i (per-partition scale) and
  H_i = A_i.T @ trkT via 2 accumulating matmuls; relu+b1 fused on ScalarE;
  logits reduced with a shifted stationary (w2 in column r of block r) so 32
  consecutive i's accumulate into one [32,512] PSUM tile; sigmoid+b2 fused.
All host-side prep (transposes, bias folds, w2 stationary build) is numpy.
"""
import sys
import types

import numpy as np


def _install_ntff_hook():
    """Shim antenv.axon_hooks (absent on this image) so trace=True works."""
    if "antenv.axon_hooks" in sys.modules:
        return
    mod = types.ModuleType("antenv.axon_hooks")
    _hook = [None]
    mod.set_axon_ntff_profile_hook = lambda h: _hook.__setitem__(0, h)
    mod.get_axon_ntff_profile_hook = lambda: _hook[0]
    sys.modules["antenv.axon_hooks"] = mod
    try:
        from trn_agent_boot.trn_boot import _ntff_profile_via_ctypes
        mod.set_axon_ntff_profile_hook(
            _ntff_profile_via_ctypes("/opt/axon/libaxon_pjrt.so"))
    except Exception:
        pass


_install_ntff_hook()

import concourse.bacc as bacc  # noqa: E402
import concourse.mybir as mybir  # noqa: E402
import concourse.tile as tile  # noqa: E402
from concourse.bass_utils import run_bass_kernel_spmd  # noqa: E402

F32 = mybir.dt.float32
F32R = mybir.dt.float32r
AF = mybir.ActivationFunctionType

B, ND, NT, D = 4, 512, 512, 256
H, DK = 8, 32
DHID = 128
NDC = 256          # detections per core
LN_EPS = 1e-5
N_CORES = 8
GROUP = 32         # detections per logits/sigmoid group

_CACHE = {}


def _build():
    nc = bacc.Bacc("TRN2", target_bir_lowering=False, debug=False)

    def din(name, shape):
        return nc.dram_tensor(name, shape, F32, kind="ExternalInput").ap()

    detT = din("detT", [D, NDC])        # det_chunk.T
    det_bo = din("det_bo", [NDC, D])    # det_chunk + b_o
    trkT = din("trkT", [D, NT])         # tracks[b].T
    wqT = din("wqT", [D, D])
    wkT = din("wkT", [D, D])
    wvT = din("wvT", [D, D])
    woT = din("woT", [D, D])
    bq = din("bq", [D])
    bk = din("bk", [D])
    bv = din("bv", [D])
    lng = din("lng", [D])
    lnb = din("lnb", [D])
    w1T = din("w1T", [D, DHID])
    b1 = din("b1", [DHID])
    w2s = din("w2s", [DHID, GROUP * GROUP])  # shifted stationary blocks
    b2b = din("b2b", [GROUP])
    ident = din("ident", [128, 128])
    out = nc.dram_tensor("out", [NDC, NT], F32, kind="ExternalOutput").ap()

    with tile.TileContext(nc) as tc:
        with (
            tc.tile_pool(name="persist", bufs=1) as pp,
            tc.tile_pool(name="stage", bufs=1) as stg,
        ):
            # ---- load + round inputs ----
            def load_r(ap, p, f, nt):
                """DMA f32 [nt*128, f] dram -> nt staging tiles -> f32r tiles."""
                outs = []
                nm = ap.tensor.name
                for t in range(nt):
                    s = stg.tile([p, f], F32, tag=f"stg_{nm}_{t}",
                                 name=f"stg_{nm}_{t}")
                    nc.sync.dma_start(s[:], ap[t * p:(t + 1) * p, :])
                    r = pp.tile([p, f], F32R, tag=f"r_{nm}_{t}",
                                name=f"r_{nm}_{t}")
                    nc.vector.tensor_copy(r[:], s[:])
                    outs.append(r)
                return outs

            trkT_r = load_r(trkT, 128, NT, 2)
            detT_r = load_r(detT, 128, NDC, 2)
            wqT_r = load_r(wqT, 128, D, 2)
            wkT_r = load_r(wkT, 128, D, 2)
            wvT_r = load_r(wvT, 128, D, 2)
            woT_r = load_r(woT, 128, D, 2)
            w1T_r = load_r(w1T, 128, DHID, 2)
            w2s_r = load_r(w2s, 128, GROUP * GROUP, 1)[0]

            det_bo_t = []
            for t in range(2):
                s = pp.tile([128, D], F32, tag=f"det_bo_{t}",
                            name=f"det_bo_{t}")
                nc.sync.dma_start(s[:], det_bo[t * 128:(t + 1) * 128, :])
                det_bo_t.append(s)

            idn = pp.tile([128, 128], F32)
            nc.sync.dma_start(idn[:], ident[:])

            def vec_tiles(ap, n=2):
                ts = []
                nm = ap.tensor.name
                for t in range(n):
                    s = pp.tile([128, 1], F32, tag=f"v_{nm}_{t}",
                                name=f"v_{nm}_{t}")
                    nc.sync.dma_start(s[:, 0], ap[t * 128:(t + 1) * 128])
                    ts.append(s)
                return ts

            bv_t = vec_tiles(bv)
            lng_t = vec_tiles(lng)
            lnb_t = vec_tiles(lnb)
            b1_t = vec_tiles(b1, 1)[0]
            b2_t = pp.tile([GROUP, 1], F32)
            nc.sync.dma_start(b2_t[:, 0], b2b[:])
            # per-head bias tiles at partition base 0
            bq_h = []
            bk_h = []
            for h in range(H):
                s = pp.tile([DK, 1], F32, tag=f"bq_{h}", name=f"bq_{h}")
                nc.sync.dma_start(s[:, 0], bq[h * DK:(h + 1) * DK])
                bq_h.append(s)
                s = pp.tile([DK, 1], F32, tag=f"bk_{h}", name=f"bk_{h}")
                nc.sync.dma_start(s[:, 0], bk[h * DK:(h + 1) * DK])
                bk_h.append(s)

            eps_t = pp.tile([128, 1], F32)
            nc.vector.memset(eps_t[:], LN_EPS)
            ones_f = pp.tile([128, 1], F32)
            nc.vector.memset(ones_f[:], 1.0)
            ones_r = pp.tile([128, 1], F32R)
            nc.vector.tensor_copy(ones_r[:], ones_f[:])

            # ---- projections (per head, feature-major) ----
            qh = []   # [DK, NDC] f32r per head
            kh = []   # [DK, NT] f32r per head
            v_sb = []  # token-major V [128 j, 256 dv] f32r per j-chunk
            with tc.tile_pool(name="proj_ps", bufs=2, space="PSUM") as pps:
                for h in range(H):
                    t, sl = divmod(h, 4)
                    ps = pps.tile([DK, NDC], F32, tag="q")
                    for dc in range(2):
                        nc.tensor.matmul(
                            ps[:], wqT_r[dc][:, h * DK:(h + 1) * DK],
                            detT_r[dc][:], start=(dc == 0), stop=(dc == 1))
                    q = pp.tile([DK, NDC], F32R, tag=f"qh_{h}",
                                name=f"qh_{h}")
                    nc.scalar.activation(q[:], ps[:], AF.Identity,
                                         bias=bq_h[h][:])
                    qh.append(q)

                    ps = pps.tile([DK, NT], F32, tag="k")
                    for dc in range(2):
                        nc.tensor.matmul(
                            ps[:], wkT_r[dc][:, h * DK:(h + 1) * DK],
                            trkT_r[dc][:], start=(dc == 0), stop=(dc == 1))
                    k = pp.tile([DK, NT], F32R, tag=f"kh_{h}",
                                name=f"kh_{h}")
                    nc.scalar.activation(k[:], ps[:], AF.Identity,
                                         bias=bk_h[h][:])
                    kh.append(k)

                ones8 = pp.tile([128, H], F32)
                nc.vector.memset(ones8[:], 1.0)
                zero8 = pp.tile([128, H], F32)
                nc.vector.memset(zero8[:], 0.0)
                for jc in range(4):
                    ps = pps.tile([128, D], F32, tag="v")
                    for dc in range(2):
                        nc.tensor.matmul(
                            ps[:], trkT_r[dc][:, jc * 128:(jc + 1) * 128],
                            wvT_r[dc][:], start=(dc == 0), stop=(dc == 1))
                    v = pp.tile([128, H * 34], F32R, tag=f"vsb_{jc}",
                                name=f"vsb_{jc}")
                    vr = v.rearrange("p (h c) -> p h c", c=34)
                    nc.vector.tensor_copy(
                        vr[:, :, 0:32], ps.rearrange("p (h c) -> p h c", c=32))
                    nc.vector.tensor_copy(
                        vr[:, :, 32:33],
                        ones8.rearrange("p (h o) -> p h o", o=1))
                    nc.vector.tensor_copy(
                        vr[:, :, 33:34],
                        zero8.rearrange("p (h o) -> p h o", o=1))
                    v_sb.append(v)

            # ---- attention: scores -> exp -> ctx/sums ----
            inv_sqrt_dk = 1.0 / np.sqrt(DK)
            with (
                tc.tile_pool(name="ctx_ps", bufs=1, space="PSUM") as cps,
                tc.tile_pool(name="eh_sb", bufs=3) as esb,
            ):
                psum_ctx = [cps.tile([128, H * 34], F32, tag=f"ctx{ic}",
                                     name=f"psum_ctx{ic}") for ic in range(2)]
                with tc.tile_pool(name="s_ps", bufs=2, space="PSUM") as sps:
                    for h in range(H):
                        for jc in range(4):
                            ps = sps.tile([128, NDC], F32, tag="s")
                            nc.tensor.matmul(
                                ps[:], kh[h][:, jc * 128:(jc + 1) * 128],
                                qh[h][:], start=True, stop=True)
                            e = esb.tile([128, NDC], F32R, tag=f"e{jc}")
                            nc.scalar.activation(e[:], ps[:], AF.Exp,
                                                 scale=inv_sqrt_dk)
                            for ic in range(2):
                                nc.tensor.matmul(
                                    psum_ctx[ic][:, h * 34:(h + 1) * 34],
                                    e[:, ic * 128:(ic + 1) * 128],
                                    v_sb[jc][:, h * 34:(h + 1) * 34],
                                    start=(jc == 0), stop=(jc == 3))

                # normalize ctx (token-major), transpose, +b_v
                recip = pp.tile([128, 2 * H], F32)
                for ic in range(2):
                    for h in range(H):
                        nc.vector.reciprocal(
                            recip[:, ic * H + h:ic * H + h + 1],
                            psum_ctx[ic][:, h * 34 + 32:h * 34 + 33])
                ctx_sb = []
                for ic in range(2):
                    c = pp.tile([128, D], F32, tag=f"ctx_sb_{ic}",
                                name=f"ctx_sb_{ic}")
                    for h in range(H):
                        nc.vector.tensor_scalar_mul(
                            c[:, h * DK:(h + 1) * DK],
                            psum_ctx[ic][:, h * 34:h * 34 + 32],
                            recip[:, ic * H + h:ic * H + h + 1])
                    ctx_sb.append(c)

            ctxT = [pp.tile([128, NDC], F32R, tag=f"ctxT{dc}",
                            name=f"ctxT{dc}") for dc in range(2)]
            with tc.tile_pool(name="tr_ps", bufs=2, space="PSUM") as tps:
                for ic in range(2):
                    for dc in range(2):
                        pt = tps.tile([128, 128], F32, tag="tr")
                        nc.tensor.transpose(
                            pt[:], ctx_sb[ic][:, dc * 128:(dc + 1) * 128],
                            idn[:])
                        nc.scalar.activation(
                            ctxT[dc][:, ic * 128:(ic + 1) * 128], pt[:],
                            AF.Identity, bias=bv_t[dc][:])

                # ---- attended + residual + LayerNorm ----
                xnT = [[pp.tile([128, 128], F32, tag=f"xnT{dc}_{ic}",
                                name=f"xnT{dc}_{ic}") for ic in range(2)]
                       for dc in range(2)]
                with tc.tile_pool(name="ln_ps", bufs=2, space="PSUM") as lps:
                    for ic in range(2):
                        ps = lps.tile([128, D], F32, tag="att")
                        for dc in range(2):
                            nc.tensor.matmul(
                                ps[:], ctxT[dc][:, ic * 128:(ic + 1) * 128],
                                woT_r[dc][:], start=(dc == 0), stop=(dc == 1))
                        x = stg.tile([128, D], F32, tag="x")
                        nc.vector.tensor_add(x[:], ps[:], det_bo_t[ic][:])
                        # stats
                        ssum = stg.tile([128, 1], F32, tag="ssum")
                        nc.vector.reduce_sum(ssum[:], x[:],
                                             axis=mybir.AxisListType.X)
                        mu = stg.tile([128, 1], F32, tag="mu")
                        nc.vector.tensor_scalar_mul(mu[:], ssum[:], 1.0 / D)
                        sq = stg.tile([128, D], F32, tag="sq")
                        ssq = stg.tile([128, 1], F32, tag="ssq")
                        nc.scalar.activation(sq[:], x[:], AF.Square,
                                             accum_out=ssq[:])
                        m2 = stg.tile([128, 1], F32, tag="m2")
                        nc.vector.tensor_scalar_mul(m2[:], ssq[:], 1.0 / D)
                        mu2 = stg.tile([128, 1], F32, tag="mu2")
                        nc.vector.tensor_mul(mu2[:], mu[:], mu[:])
                        var = stg.tile([128, 1], F32, tag="var")
                        nc.vector.tensor_sub(var[:], m2[:], mu2[:])
                        sd = stg.tile([128, 1], F32, tag="sd")
                        nc.scalar.activation(sd[:], var[:], AF.Sqrt,
                                             bias=eps_t[:])
                        rstd = stg.tile([128, 1], F32, tag="rstd")
                        nc.vector.reciprocal(rstd[:], sd[:])
                        y = stg.tile([128, D], F32, tag="y")
                        nc.vector.tensor_scalar(
                            y[:], x[:], mu[:], rstd[:],
                            op0=mybir.AluOpType.subtract,
                            op1=mybir.AluOpType.mult)
                        # transpose y, apply ln scale/shift feature-major
                        for dc in range(2):
                            pt = tps.tile([128, 128], F32, tag="tr")
                            nc.tensor.transpose(
                                pt[:], y[:, dc * 128:(dc + 1) * 128], idn[:])
                            nc.vector.tensor_scalar(
                                xnT[dc][ic][:], pt[:],
                                lng_t[dc][:], lnb_t[dc][:],
                                op0=mybir.AluOpType.mult,
                                op1=mybir.AluOpType.add)

            # ---- association scorer ----
            with (
                tc.tile_pool(name="a_sb", bufs=6) as asb,
                tc.tile_pool(name="r_sb", bufs=4) as rsb,
                tc.tile_pool(name="h_ps", bufs=3, space="PSUM") as hps,
                tc.tile_pool(name="l_ps", bufs=2, space="PSUM") as lqs,
                tc.tile_pool(name="sig_sb", bufs=2) as ssb,
            ):
                for g in range(NDC // GROUP):
                    psum_l = lqs.tile([GROUP, NT], F32, tag="l")
                    for r in range(GROUP):
                        i = g * GROUP + r
                        a0 = asb.tile([128, DHID], F32R, tag="a0")
                        a1 = asb.tile([128, DHID], F32R, tag="a1")
                        ic, col = divmod(i, 128)
                        nc.vector.tensor_scalar_mul(
                            a0[:], w1T_r[0][:], xnT[0][ic][:, col:col + 1])
                        nc.vector.tensor_scalar_mul(
                            a1[:], w1T_r[1][:], xnT[1][ic][:, col:col + 1])
                        ph = hps.tile([128, NT], F32, tag="h")
                        nc.tensor.matmul(ph[:], a0[:], trkT_r[0][:],
                                         start=True, stop=False)
                        nc.tensor.matmul(ph[:], a1[:], trkT_r[1][:],
                                         start=False, stop=True)
                        rt = rsb.tile([128, NT], F32R, tag="r")
                        nc.scalar.activation(rt[:], ph[:], AF.Relu,
                                             bias=b1_t[:])
                        nc.tensor.matmul(
                            psum_l[:], w2s_r[:, r * GROUP:(r + 1) * GROUP],
                            rt[:], start=(r == 0), stop=(r == GROUP - 1))
                    sg = ssb.tile([GROUP, NT], F32, tag="sig")
                    nc.scalar.activation(sg[:], psum_l[:], AF.Sigmoid,
                                         bias=b2_t[:])
                    nc.sync.dma_start(
                        out[g * GROUP:(g + 1) * GROUP, :], sg[:])

    nc.compile()
    return nc


def _host_prep(inputs):
    """Build the 8 per-core input maps from full inputs (numpy, cheap)."""
    det = np.ascontiguousarray(inputs["detections"], np.float32)
    trk = np.ascontiguousarray(inputs["tracks"], np.float32)
    f32 = lambda x: np.ascontiguousarray(np.asarray(x), np.float32)
    w_q, b_q = f32(inputs["w_q"]), f32(inputs["b_q"])
    w_k, b_k = f32(inputs["w_k"]), f32(inputs["b_k"])
    w_v, b_v = f32(inputs["w_v"]), f32(inputs["b_v"])
    w_o, b_o = f32(inputs["w_o"]), f32(inputs["b_o"])
    ln_g, ln_b = f32(inputs["ln_g"]), f32(inputs["ln_b"])
    w1, b1 = f32(inputs["w1"]), f32(inputs["b1"])
    w2, b2 = f32(inputs["w2"]), f32(inputs["b2"])

    w2s = np.zeros((DHID, GROUP * GROUP), np.float32)
    for r in range(GROUP):
        w2s[:, r * GROUP + r] = w2[0]
    shared = {
        "wqT": np.ascontiguousarray(w_q.T), "wkT": np.ascontiguousarray(w_k.T),
        "wvT": np.ascontiguousarray(w_v.T), "woT": np.ascontiguousarray(w_o.T),
        "bq": b_q, "bk": b_k, "bv": b_v,
        "lng": ln_g, "lnb": ln_b,
        "w1T": np.ascontiguousarray(w1.T), "b1": b1,
        "w2s": w2s, "b2b": np.full(GROUP, b2[0], np.float32),
        "ident": np.eye(128, dtype=np.float32),
    }
    in_maps = []
    for c in range(N_CORES):
        b, half = divmod(c, 2)
        dchunk = det[b, half * NDC:(half + 1) * NDC, :]
        m = dict(shared)
        m["detT"] = np.ascontiguousarray(dchunk.T)
        m["det_bo"] = np.ascontiguousarray(dchunk + b_o[None, :])
        m["trkT"] = np.ascontiguousarray(trk[b].T)
        in_maps.append(m)
    return in_maps


def _get_nc():
    if "nc" not in _CACHE:
        _CACHE["nc"] = _build()
    return _CACHE["nc"]


def run(inputs, trace=False):
    nc = _get_nc()
    in_maps = _host_prep(inputs)
    res = run_bass_kernel_spmd(nc, in_maps, core_ids=list(range(N_CORES)),
                               trace=trace)
    full = np.empty((B, ND, NT), np.float32)
    for c in range(N_CORES):
        b, half = divmod(c, 2)
        full[b, half * NDC:(half + 1) * NDC, :] = res.results[c]["out"]
    return full, res


def kernel(**inputs):
    return run(inputs, trace=False)[0]

